# revision 30
# baseline (speedup 1.0000x reference)
"""Trainium2 Bass kernel for nn_CausalSelfAttention_2860448219236.

Reference semantics (B=2, S=2048, H=1024, NH=16, HD=64, WINDOW=512, NEG=-1e4):
  q/k/v = heads(hs @ W{q,k,v}.T + b)
  mask  = causal(j>i: NEG) + window(j >= i-512: NEG) + attention_mask
  out   = softmax(q k^T/8 + mask) v

Because NEG=-1e4 and softmax subtracts the row max, the f32 result equals a
*binary*-masked softmax over the allowed set
  A(i) = {j <= i}        for i <= 512  (whole row carries the same -1e4)
       = {j <= i-513}    for i >= 513  (recent-window entries underflow to 0)

Sharding: core c = (batch b = c//4) x (head group g = c%4, heads 4g..4g+3).
Fully data-parallel SPMD - one program, per-core input slices, no collectives.

Per-core pipeline (all bulk matmuls fp32r, ~1.5e-4 relerr):
  scoresT[s,t] layout (no transposes needed anywhere in the main grid):
    qT[e,t], kT[e,s] = W_aug^T-tiles @ hsT_aug  (ones row in hsT_aug folds
    biases; elided entirely when all biases are zero); v_aug[s, 4*(64+1)]
    with a ones column per head -> Z fused into the PV matmul as row 64
    scoresT = kT^T qT      (2 heads packed per MM via row tile_position)
    E = exp(scale*scores + attn_mask_bias)  (ACT; per-partition bias = per-s)
    boundary masking: multiply E by 0/1 diagonal masks (GPSIMD) on the two
      slope-1 boundary diagonals only; dead regions are never computed (QK/
      exp/PV all run on the alive column subrange)
    out = pv[0:64] * 1/pv[64]  (fast reciprocal + partition-broadcast DMA)
  Input DMA is section-ordered (exactly what the next projection group
  needs), and all projections are emitted inside pair-0's attention so the
  ACT engine never starves during pair-1.  t=512 (the one output column
  whose window boundary is not block-aligned) is recomputed exactly in a
  small batched row-layout pass at the end.
"""

import numpy as np

S = 2048
H = 1024
B = 2
NH = 16
HD = 64
WINDOW = 512
SCALE = 0.125
SB = 128          # s block
TC = 512          # t chunk
NTC = S // TC     # 4
NSB = S // SB     # 16
NHC = 4           # heads per core
KTS = S - 512     # kT/v s-extent needed by the main grid (1536)
NVT = KTS // SB   # 12 v tiles
VW = NHC * (HD + 1)  # 260
WCOLS = 2 * 256 + VW  # q(256) k(256) v_aug(260) = 772
HP = H + 1        # augmented contraction (ones row)

_CACHE = {}


def _alive_sbs(tci):
    if tci == 0:
        return list(range(4))
    return list(range(min(4 * tci, NVT)))


def _x_lo(sb, tci):
    first_tb = sb if tci == 0 else sb + 4
    return max(0, SB * (first_tb - 4 * tci))


def _diag_actions(sb, tci):
    """[(block_in_chunk, mask_idx, col_off)]; mask 0 = p<=x, 1 = p<=x-1."""
    acts = []
    for tb in range(4 * tci, 4 * tci + 4):
        if tb <= 3 and tb == sb:
            acts.append((tb - 4 * tci, 0, 0))
        if tb >= 4 and tb - 4 == sb:
            # at t0=512 (tci==1, block 0) leave col 0 unmasked: that column
            # (t=512) is recomputed by the special pass, and masking it fully
            # would make Z=0 -> NaN noise.
            col_off = 1 if (tci == 1 and tb == 4) else 0
            acts.append((tb - 4 * tci, 1, col_off))
    return acts


def _build_program(with_bias=False, no_special=False, no_norm=False, psum_split=False, emission='front'):
    import concourse.bass as bass_mod
    import concourse.bacc as bacc
    import concourse.mybir as mybir
    from concourse.tile import TileContext

    F32 = mybir.dt.float32
    F32R = mybir.dt.float32r
    EXP = mybir.ActivationFunctionType.Exp

    nc = bacc.Bacc("TRN2", target_bir_lowering=False, debug=False)

    hst_d = nc.dram_tensor("hst", [HP, S], F32R, kind="ExternalInput")
    w_d = nc.dram_tensor("w", [HP, WCOLS], F32R, kind="ExternalInput")
    AUXW = NSB + 2 * SB + 4 + S  # attc | masks | eye4 | attr(row 0)
    aux_d = nc.dram_tensor("aux", [SB, AUXW], F32, kind="ExternalInput")
    outT_d = nc.dram_tensor("outT", [NHC * HD, S], F32, kind="ExternalOutput")
    scr_d = nc.dram_tensor("scr", [NHC, 640], F32)  # special-pass scratch

    NKT = H // SB        # 8 full k tiles
    NKA = NKT + (1 if with_bias else 0)  # tiles incl. optional bias row
    kp = lambda k: SB if k < NKT else 1  # partition count of k-tile

    with TileContext(nc) as tc:
        with tc.tile_pool(name="stat", bufs=1) as stat:
            qt = [stat.tile([SB, S], F32R, tag=f"qt{e}", name=f"qt{e}")
                  for e in range(2)]
            kt = [stat.tile([SB, KTS], F32R, tag=f"kt{e}", name=f"kt{e}")
                  for e in range(2)]
            vt = [stat.tile([SB, VW], F32R, tag=f"vt{i}", name=f"vt{i}")
                  for i in range(NVT)]
            hst = [stat.tile([kp(k), S], F32R, tag=f"hst{k}", name=f"hst{k}")
                   for k in range(NKA)]
            wt = [stat.tile([kp(k), WCOLS], F32R, tag=f"wt{k}", name=f"wt{k}")
                  for k in range(NKA)]
            aux = stat.tile([SB, AUXW], F32)
            attc = aux[:, 0:NSB]
            masks_off = NSB  # masks AP built from aux tensor directly
            eye4 = aux[0:4, NSB + 2 * SB : NSB + 2 * SB + 4]
            attr = aux[0:1, NSB + 2 * SB + 4 : AUXW]

            # --- section-ordered input DMA (w cols are [q0|k0|v|q1|k1]) ---
            for k in range(NKA):  # q0+k0 weight cols, then hsT c0
                nc.sync.dma_start(out=wt[k][:, 0:256],
                                  in_=w_d[k * SB : k * SB + kp(k), 0:256])
                nc.sync.dma_start(out=hst[k][:, 0:TC],
                                  in_=hst_d[k * SB : k * SB + kp(k), 0:TC])
            nc.sync.dma_start(out=aux[:], in_=aux_d[:])
            for k in range(NKA):  # v_aug weight cols
                nc.sync.dma_start(
                    out=wt[k][:, 256 : 256 + VW],
                    in_=w_d[k * SB : k * SB + kp(k), 256 : 256 + VW],
                )
            for k in range(NKA):  # hsT c1
                nc.sync.dma_start(
                    out=hst[k][:, TC : 2 * TC],
                    in_=hst_d[k * SB : k * SB + kp(k), TC : 2 * TC],
                )
            for cc in (2, 3):  # hsT c2, then c3
                for k in range(NKA):
                    nc.sync.dma_start(
                        out=hst[k][:, cc * TC : (cc + 1) * TC],
                        in_=hst_d[k * SB : k * SB + kp(k),
                                  cc * TC : (cc + 1) * TC],
                    )
            for k in range(NKA):  # pair-1 weight cols
                nc.sync.dma_start(
                    out=wt[k][:, 256 + VW : WCOLS],
                    in_=w_d[k * SB : k * SB + kp(k), 256 + VW : WCOLS],
                )

            with (
                tc.tile_pool(name="mmps", bufs=(2 if psum_split else 3),
                             space="PSUM") as mmps,
                tc.tile_pool(name="ppps", bufs=2, space="PSUM") as ppps,
                tc.tile_pool(name="pvps", bufs=2, space="PSUM") as pvpool,
                tc.tile_pool(name="epool", bufs=4) as epool,
                tc.tile_pool(name="wpool", bufs=3) as wpool,
                tc.tile_pool(name="ssb", bufs=1) as ssb,
            ):

                def proj_tile():
                    if psum_split:
                        return ppps.tile([SB, TC], F32, tag="pp", name="pp")
                    return mmps.tile([SB, 2 * TC], F32, tag="mm", name="pp")

                W_Q = (0, 516)
                W_K = (128, 644)

                def emit_q(e, tcc):
                    pp = proj_tile()
                    for k in range(NKA):
                        nc.tensor.matmul(
                            pp[:, 0:TC],
                            wt[k][:, W_Q[e] : W_Q[e] + SB],
                            hst[k][:, tcc * TC : (tcc + 1) * TC],
                            start=(k == 0),
                            stop=(k == NKA - 1),
                        )
                    nc.scalar.copy(
                        qt[e][:, tcc * TC : (tcc + 1) * TC], pp[:, 0:TC]
                    )

                def emit_k(e, scc):
                    pp = proj_tile()
                    for k in range(NKA):
                        nc.tensor.matmul(
                            pp[:, 0:TC],
                            wt[k][:, W_K[e] : W_K[e] + SB],
                            hst[k][:, scc * TC : (scc + 1) * TC],
                            start=(k == 0),
                            stop=(k == NKA - 1),
                        )
                    nc.scalar.copy(
                        kt[e][:, scc * TC : (scc + 1) * TC], pp[:, 0:TC]
                    )

                def emit_v(sb):
                    pp = proj_tile()
                    for k in range(NKA):
                        nc.tensor.matmul(
                            pp[:, 0:VW],
                            hst[k][:, sb * SB : (sb + 1) * SB],
                            wt[k][:, 256 : 256 + VW],
                            start=(k == 0),
                            stop=(k == NKA - 1),
                        )
                    nc.scalar.copy(vt[sb][:], pp[:, 0:VW])
                    if not with_bias:
                        # ones-selector columns never got their 1.0 (it lives
                        # in the elided bias row): write them directly.
                        for j in range(NHC):
                            nc.vector.memset(
                                vt[sb][:, j * (HD + 1) + HD :
                                       j * (HD + 1) + HD + 1].bitcast(F32),
                                1.0,
                            )

                def emit_attn(pair, tci):
                    sbs = _alive_sbs(tci)
                    pv_acc = [
                        pvpool.tile([HD + 1, TC], F32, tag="pvacc", name="pvacc")
                        for _ in range(2)
                    ]
                    for i, sb in enumerate(sbs):
                        xlo = _x_lo(sb, tci)
                        first, last = (i == 0), (i == len(sbs) - 1)
                        pqk = mmps.tile([SB, 2 * TC], F32, tag="mm", name="pqk")
                        for h2 in range(2):
                            nc.tensor.matmul(
                                pqk[:, h2 * TC + xlo : (h2 + 1) * TC],
                                kt[pair][h2 * HD : (h2 + 1) * HD,
                                         sb * SB : (sb + 1) * SB],
                                qt[pair][h2 * HD : (h2 + 1) * HD,
                                         tci * TC + xlo : (tci + 1) * TC],
                                start=True,
                                stop=True,
                                tile_position=(h2 * HD, 0),
                            )
                        te = epool.tile([SB, 2 * TC], F32R, tag="te", name="te")
                        te3 = te[:].rearrange("p (b n) -> p b n", b=2)
                        pqk3 = pqk[:].rearrange("p (b n) -> p b n", b=2)
                        nc.scalar.activation(
                            te3[:, :, xlo:TC],
                            pqk3[:, :, xlo:TC],
                            EXP,
                            bias=attc[:, sb : sb + 1],
                            scale=SCALE,
                        )
                        for blk, mi, coff in _diag_actions(sb, tci):
                            x = blk * SB + coff
                            m_ap = bass_mod.AP(
                                tensor=aux[:].tensor,
                                offset=NSB + mi * SB + coff,
                                ap=[[AUXW, SB], [0, 2], [1, SB - coff]],
                            )
                            nc.gpsimd.tensor_mul(
                                te3[:, :, x : blk * SB + SB],
                                te3[:, :, x : blk * SB + SB],
                                m_ap,
                            )
                        for h2 in range(2):
                            hl = 2 * pair + h2
                            nc.tensor.matmul(
                                pv_acc[h2][:, xlo:TC],
                                vt[sb][:, hl * (HD + 1) : (hl + 1) * (HD + 1)],
                                te[:, h2 * TC + xlo : (h2 + 1) * TC],
                                start=first,
                                stop=last,
                            )
                    # copy psum->SBUF first (releases pvacc banks ~2.5us
                    # earlier than normalizing straight from PSUM), then
                    # pair-combined normalize: 1 bcast DMA + 1 output DMA
                    pvc = wpool.tile([HD + 1, 2 * TC], F32, tag="pvc",
                                     name="pvc")
                    for h2 in range(2):
                        nc.vector.tensor_copy(
                            pvc[:, h2 * TC : (h2 + 1) * TC], pv_acc[h2][:]
                        )
                    rr2 = wpool.tile([SB, TC], F32, tag="rr", name="rr2")
                    for h2 in range(2):
                        nc.vector.reciprocal(
                            rr2[HD * h2 : HD * h2 + 1, :],
                            pvc[HD : HD + 1, h2 * TC : (h2 + 1) * TC],
                        )
                    rrep2 = wpool.tile([HD, 2 * TC], F32, tag="rrep", name="rrep2")
                    for h2 in range(2):
                        src_b = bass_mod.AP(
                            tensor=rr2[:].tensor,
                            offset=HD * h2 * TC,
                            ap=[[TC, 1], [0, HD], [1, TC]],
                        )
                        nc.scalar.dma_start(
                            out=rrep2[:, h2 * TC : (h2 + 1) * TC], in_=src_b
                        )
                    osb2 = wpool.tile([HD, 2 * TC], F32, tag="osb", name="osb2")
                    for h2 in range(2):
                        nc.vector.tensor_mul(
                            osb2[:, h2 * TC : (h2 + 1) * TC],
                            pvc[0:HD, h2 * TC : (h2 + 1) * TC],
                            rrep2[:, h2 * TC : (h2 + 1) * TC],
                        )
                    for h2 in range(2):
                        hl = 2 * pair + h2
                        x0 = 1 if tci == 1 else 0
                        nc.scalar.dma_start(
                            out=outT_d[hl * HD : (hl + 1) * HD,
                                       tci * TC + x0 : (tci + 1) * TC],
                            in_=osb2[:, h2 * TC + x0 : (h2 + 1) * TC],
                        )

                def emit_special_probs():
                    """t=512 probs row per head -> erT4 [128, 5] PV layout.

                    Head hl lives at partition 32*hl (legal AP base); the
                    normalized probs row reaches PV layout via a DRAM
                    round-trip scatter.  Emitted before pair-1 attention so
                    the whole latency chain hides under it.
                    """
                    for hl in range(NHC):
                        pair, half = hl // 2, hl % 2
                        qcol = qt[pair][half * HD : (half + 1) * HD, 512:513]
                        pr = mmps.tile([SB, 2 * TC], F32, tag="mm", name="pr")
                        nc.tensor.matmul(
                            pr[0:1, 0:512], qcol,
                            kt[pair][half * HD : (half + 1) * HD, 0:512],
                            start=True, stop=True,
                        )
                        nc.tensor.matmul(
                            pr[0:1, 512:513], qcol.bitcast(F32),
                            kt[pair][half * HD : (half + 1) * HD, 512:513]
                            .bitcast(F32),
                            start=True, stop=True,
                        )
                        # attn-mask (host pre-scaled by 1/SCALE)
                        nc.vector.tensor_add(
                            pr[0:1, 0:513], pr[0:1, 0:513], attr[0:1, 0:513]
                        )
                        er = ssb.tile([1, 513], F32, tag=f"er{hl}",
                                      name=f"er{hl}")
                        z = ssb.tile([1, 1], F32, tag=f"z{hl}", name=f"z{hl}")
                        rz = ssb.tile([1, 1], F32, tag=f"rz{hl}",
                                      name=f"rz{hl}")
                        nc.scalar.activation(
                            er[:], pr[0:1, 0:513], EXP,
                            scale=SCALE, accum_out=z[:],
                        )
                        nc.vector.reciprocal(rz[:], z[:])
                        nc.vector.tensor_scalar_mul(er[:], er[:], rz[:])
                        nc.scalar.dma_start(out=scr_d[hl, 0:513], in_=er[:])
                        src_t = bass_mod.AP(
                            tensor=scr_d.ap().tensor,
                            offset=hl * 640,
                            ap=[[1, SB], [SB, 5]],
                        )
                        nc.scalar.dma_start(
                            out=erT4[:, 5 * hl : 5 * hl + 5], in_=src_t
                        )

                def emit_special_pv():
                    po4_t = pvpool.tile([HD + 1, TC], F32, tag="pvacc", name="po4")
                    po4 = po4_t[0:HD, 0:NHC]
                    for hl in range(NHC):
                        for sb4 in range(4):
                            nc.tensor.matmul(
                                po4[:, hl : hl + 1],
                                vt[sb4][:, hl * (HD + 1) : hl * (HD + 1) + HD]
                                .bitcast(F32),
                                erT4[:, 5 * hl + sb4 : 5 * hl + sb4 + 1],
                                start=(sb4 == 0),
                                stop=False,
                            )
                        nc.tensor.matmul(
                            po4[:, hl : hl + 1],
                            vt[4][0:1, hl * (HD + 1) : hl * (HD + 1) + HD]
                            .bitcast(F32),
                            erT4[0:1, 5 * hl + 4 : 5 * hl + 5],
                            start=False,
                            stop=True,
                        )
                    o54 = ssb.tile([HD, NHC], F32, tag="o54", name="o54")
                    nc.vector.tensor_copy(o54[:], po4)
                    for hl in range(NHC):
                        nc.scalar.dma_start(
                            out=outT_d[hl * HD : (hl + 1) * HD, 512:513],
                            in_=o54[:, hl : hl + 1],
                        )

                erT4 = ssb.tile([SB, 5 * NHC], F32, tag="erT4", name="erT4")

                if emission == "front":
                    emit_q(0, 0); emit_k(0, 0)
                    for sb in range(4):
                        emit_v(sb)
                    emit_attn(0, 0)
                    emit_q(0, 1); emit_k(0, 1)
                    for sb in range(4, 8):
                        emit_v(sb)
                    emit_attn(0, 1)
                    emit_q(0, 2); emit_k(0, 2)
                    for sb in range(8, NVT):
                        emit_v(sb)
                    emit_attn(0, 2)
                    emit_q(0, 3); emit_q(1, 0); emit_k(1, 0)
                    emit_attn(0, 3)
                    emit_q(1, 1); emit_q(1, 2); emit_q(1, 3)
                    emit_k(1, 1); emit_k(1, 2)
                    if not no_special:
                        emit_special_probs()
                    emit_attn(1, 0); emit_attn(1, 1)
                    emit_attn(1, 2); emit_attn(1, 3)
                    if not no_special:
                        emit_special_pv()
                else:  # paced
                    emit_q(0, 0); emit_k(0, 0)
                    for sb in range(4):
                        emit_v(sb)
                    emit_attn(0, 0)
                    emit_q(0, 1)
                    emit_attn(0, 1)
                    emit_q(0, 2); emit_k(0, 1)
                    for sb in range(4, 8):
                        emit_v(sb)
                    emit_attn(0, 2)
                    emit_q(0, 3); emit_k(0, 2)
                    for sb in range(8, NVT):
                        emit_v(sb)
                    emit_q(1, 0); emit_k(1, 0)
                    emit_attn(0, 3)
                    emit_q(1, 1); emit_q(1, 2)
                    emit_attn(1, 0)
                    emit_q(1, 3); emit_k(1, 1)
                    emit_attn(1, 1)
                    emit_k(1, 2)
                    emit_attn(1, 2)
                    emit_attn(1, 3)
                    if not no_special:
                        emit_special_probs()
                        emit_special_pv()

    nc.compile()
    return nc


def _host_prep(inputs, with_bias):
    hs = np.asarray(inputs["hidden_states"], dtype=np.float32)
    am = np.asarray(inputs["attention_mask"], dtype=np.float32)
    Wq = np.asarray(inputs["Wq"], dtype=np.float32)
    bq = np.asarray(inputs["bq"], dtype=np.float32)
    Wk = np.asarray(inputs["Wk"], dtype=np.float32)
    bk = np.asarray(inputs["bk"], dtype=np.float32)
    Wv = np.asarray(inputs["Wv"], dtype=np.float32)
    bv = np.asarray(inputs["bv"], dtype=np.float32)

    p = np.arange(SB)[:, None]
    x = np.arange(SB)[None, :]
    m0 = (p <= x).astype(np.float32)
    m1 = (p <= x - 1).astype(np.float32)
    masks = np.concatenate([m0, m1], axis=1)
    eye4 = np.zeros((SB, 4), dtype=np.float32)
    eye4[:4] = np.eye(4, dtype=np.float32)

    in_maps = []
    for c in range(8):
        b, g = c // 4, c % 4
        hsT_aug = np.concatenate(
            [hs[b].T, np.ones((1, S), dtype=np.float32)], axis=0
        )
        # w cols: [q-e0 | k-e0 | v_aug | q-e1 | k-e1]
        w = np.zeros((HP, WCOLS), dtype=np.float32)
        hsl = slice(256 * g, 256 * (g + 1))
        WqT = Wq[hsl, :].T
        WkT = Wk[hsl, :].T
        w[:H, 0:128] = WqT[:, 0:128]
        w[H, 0:128] = bq[hsl][0:128]
        w[:H, 516:644] = WqT[:, 128:256]
        w[H, 516:644] = bq[hsl][128:256]
        w[:H, 128:256] = WkT[:, 0:128]
        w[H, 128:256] = bk[hsl][0:128]
        w[:H, 644:772] = WkT[:, 128:256]
        w[H, 644:772] = bk[hsl][128:256]
        for j in range(NHC):
            cs = slice(256 * g + HD * j, 256 * g + HD * (j + 1))
            w[:H, 256 + 65 * j : 256 + 65 * j + HD] = Wv[cs, :].T
            w[H, 256 + 65 * j : 256 + 65 * j + HD] = bv[cs]
            w[H, 256 + 65 * j + HD] = 1.0  # ones-selector column
        amv = am[b, 0, 0, :].astype(np.float32)
        attc = amv.reshape(NSB, SB).T
        attr_row = np.zeros((SB, S), dtype=np.float32)
        attr_row[0] = amv / SCALE
        aux = np.concatenate([attc, masks, eye4, attr_row], axis=1)
        in_maps.append({"hst": hsT_aug, "w": w, "aux": aux.copy()})
    return in_maps


LAST_EXEC_NS = None


def kernel(**inputs):
    import os

    from concourse.bass_utils import run_bass_kernel_spmd

    global LAST_EXEC_NS
    with_bias = bool(
        np.any(np.asarray(inputs["bq"]))
        or np.any(np.asarray(inputs["bk"]))
        or np.any(np.asarray(inputs["bv"]))
    )
    key = f"nc{int(with_bias)}"
    if key not in _CACHE:
        _CACHE[key] = _build_program(with_bias=with_bias)
    nc = _CACHE[key]
    in_maps = _host_prep(inputs, with_bias)
    trace = bool(os.environ.get("BASS_KERNEL_TRACE"))
    res = run_bass_kernel_spmd(nc, in_maps, list(range(8)), trace=trace)
    LAST_EXEC_NS = res.exec_time_ns
    out = np.empty((B, S, H), dtype=np.float32)
    for c in range(8):
        b, g = c // 4, c % 4
        out[b, :, 256 * g : 256 * (g + 1)] = res.results[c]["outT"].T
    return out


# revision 31
# speedup vs baseline: 1.0197x; 1.0197x over previous
"""Trainium2 Bass kernel for nn_CausalSelfAttention_2860448219236.

Reference semantics (B=2, S=2048, H=1024, NH=16, HD=64, WINDOW=512, NEG=-1e4):
  q/k/v = heads(hs @ W{q,k,v}.T + b)
  mask  = causal(j>i: NEG) + window(j >= i-512: NEG) + attention_mask
  out   = softmax(q k^T/8 + mask) v

Because NEG=-1e4 and softmax subtracts the row max, the f32 result equals a
*binary*-masked softmax over the allowed set
  A(i) = {j <= i}        for i <= 512  (whole row carries the same -1e4)
       = {j <= i-513}    for i >= 513  (recent-window entries underflow to 0)

Sharding: core c = (batch b = c//4) x (head group g = c%4, heads 4g..4g+3).
Fully data-parallel SPMD - one program, per-core input slices, no collectives.

Per-core pipeline (all bulk matmuls fp32r, ~1.5e-4 relerr):
  scoresT[s,t] layout (no transposes needed anywhere in the main grid):
    qT[e,t], kT[e,s] = W_aug^T-tiles @ hsT_aug  (ones row in hsT_aug folds
    biases; elided entirely when all biases are zero); v_aug[s, 4*(64+1)]
    with a ones column per head -> Z fused into the PV matmul as row 64
    scoresT = kT^T qT      (2 heads packed per MM via row tile_position)
    E = exp(scale*scores + attn_mask_bias)  (ACT; per-partition bias = per-s)
    boundary masking: multiply E by 0/1 diagonal masks (GPSIMD) on the two
      slope-1 boundary diagonals only; dead regions are never computed (QK/
      exp/PV all run on the alive column subrange)
    out = pv[0:64] * 1/pv[64]  (fast reciprocal + partition-broadcast DMA)
  Input DMA is section-ordered (exactly what the next projection group
  needs), and all projections are emitted inside pair-0's attention so the
  ACT engine never starves during pair-1.  t=512 (the one output column
  whose window boundary is not block-aligned) is recomputed exactly in a
  small batched row-layout pass at the end.
"""

import numpy as np

S = 2048
H = 1024
B = 2
NH = 16
HD = 64
WINDOW = 512
SCALE = 0.125
SB = 128          # s block
TC = 512          # t chunk
NTC = S // TC     # 4
NSB = S // SB     # 16
NHC = 4           # heads per core
KTS = S - 512     # kT/v s-extent needed by the main grid (1536)
NVT = KTS // SB   # 12 v tiles
VW = NHC * (HD + 1)  # 260
WCOLS = 2 * 256 + VW  # q(256) k(256) v_aug(260) = 772
HP = H + 1        # augmented contraction (ones row)

_CACHE = {}


def _alive_sbs(tci):
    if tci == 0:
        return list(range(4))
    return list(range(min(4 * tci, NVT)))


def _x_lo(sb, tci):
    first_tb = sb if tci == 0 else sb + 4
    return max(0, SB * (first_tb - 4 * tci))


def _diag_actions(sb, tci):
    """[(block_in_chunk, mask_idx, col_off)]; mask 0 = p<=x, 1 = p<=x-1."""
    acts = []
    for tb in range(4 * tci, 4 * tci + 4):
        if tb <= 3 and tb == sb:
            acts.append((tb - 4 * tci, 0, 0))
        if tb >= 4 and tb - 4 == sb:
            # at t0=512 (tci==1, block 0) leave col 0 unmasked: that column
            # (t=512) is recomputed by the special pass, and masking it fully
            # would make Z=0 -> NaN noise.
            col_off = 1 if (tci == 1 and tb == 4) else 0
            acts.append((tb - 4 * tci, 1, col_off))
    return acts


def _build_program(with_bias=False, no_special=False, no_norm=False, psum_split=True, emission='front'):
    import concourse.bass as bass_mod
    import concourse.bacc as bacc
    import concourse.mybir as mybir
    from concourse.tile import TileContext

    F32 = mybir.dt.float32
    F32R = mybir.dt.float32r
    EXP = mybir.ActivationFunctionType.Exp

    nc = bacc.Bacc("TRN2", target_bir_lowering=False, debug=False)

    hst_d = nc.dram_tensor("hst", [HP, S], F32R, kind="ExternalInput")
    w_d = nc.dram_tensor("w", [HP, WCOLS], F32R, kind="ExternalInput")
    AUXW = NSB + 2 * SB + 4 + S  # attc | masks | eye4 | attr(row 0)
    aux_d = nc.dram_tensor("aux", [SB, AUXW], F32, kind="ExternalInput")
    outT_d = nc.dram_tensor("outT", [NHC * HD, S], F32, kind="ExternalOutput")
    scr_d = nc.dram_tensor("scr", [NHC, 640], F32)  # special-pass scratch

    NKT = H // SB        # 8 full k tiles
    NKA = NKT + (1 if with_bias else 0)  # tiles incl. optional bias row
    kp = lambda k: SB if k < NKT else 1  # partition count of k-tile

    with TileContext(nc) as tc:
        with tc.tile_pool(name="stat", bufs=1) as stat:
            qt = [stat.tile([SB, S], F32R, tag=f"qt{e}", name=f"qt{e}")
                  for e in range(2)]
            kt = [stat.tile([SB, KTS], F32R, tag=f"kt{e}", name=f"kt{e}")
                  for e in range(2)]
            vt = [stat.tile([SB, VW], F32R, tag=f"vt{i}", name=f"vt{i}")
                  for i in range(NVT)]
            hst = [stat.tile([kp(k), S], F32R, tag=f"hst{k}", name=f"hst{k}")
                   for k in range(NKA)]
            wt = [stat.tile([kp(k), WCOLS], F32R, tag=f"wt{k}", name=f"wt{k}")
                  for k in range(NKA)]
            aux = stat.tile([SB, AUXW], F32)
            attc = aux[:, 0:NSB]
            masks_off = NSB  # masks AP built from aux tensor directly
            eye4 = aux[0:4, NSB + 2 * SB : NSB + 2 * SB + 4]
            attr = aux[0:1, NSB + 2 * SB + 4 : AUXW]

            # --- section-ordered input DMA (w cols are [q0|k0|v|q1|k1]) ---
            for k in range(NKA):  # q0+k0 weight cols, then hsT c0
                nc.sync.dma_start(out=wt[k][:, 0:256],
                                  in_=w_d[k * SB : k * SB + kp(k), 0:256])
                nc.sync.dma_start(out=hst[k][:, 0:TC],
                                  in_=hst_d[k * SB : k * SB + kp(k), 0:TC])
            nc.sync.dma_start(out=aux[:], in_=aux_d[:])
            for k in range(NKA):  # v_aug weight cols
                nc.sync.dma_start(
                    out=wt[k][:, 256 : 256 + VW],
                    in_=w_d[k * SB : k * SB + kp(k), 256 : 256 + VW],
                )
            for k in range(NKA):  # hsT c1
                nc.sync.dma_start(
                    out=hst[k][:, TC : 2 * TC],
                    in_=hst_d[k * SB : k * SB + kp(k), TC : 2 * TC],
                )
            for cc in (2, 3):  # hsT c2, then c3
                for k in range(NKA):
                    nc.sync.dma_start(
                        out=hst[k][:, cc * TC : (cc + 1) * TC],
                        in_=hst_d[k * SB : k * SB + kp(k),
                                  cc * TC : (cc + 1) * TC],
                    )
            for k in range(NKA):  # pair-1 weight cols
                nc.sync.dma_start(
                    out=wt[k][:, 256 + VW : WCOLS],
                    in_=w_d[k * SB : k * SB + kp(k), 256 + VW : WCOLS],
                )

            with (
                tc.tile_pool(name="mmps", bufs=(2 if psum_split else 3),
                             space="PSUM") as mmps,
                tc.tile_pool(name="ppps", bufs=2, space="PSUM") as ppps,
                tc.tile_pool(name="pvps", bufs=2, space="PSUM") as pvpool,
                tc.tile_pool(name="epool", bufs=4) as epool,
                tc.tile_pool(name="wpool", bufs=3) as wpool,
                tc.tile_pool(name="ssb", bufs=1) as ssb,
            ):

                def proj_tile():
                    if psum_split:
                        return ppps.tile([SB, TC], F32, tag="pp", name="pp")
                    return mmps.tile([SB, 2 * TC], F32, tag="mm", name="pp")

                W_Q = (0, 516)
                W_K = (128, 644)

                def emit_q(e, tcc):
                    pp = proj_tile()
                    for k in range(NKA):
                        nc.tensor.matmul(
                            pp[:, 0:TC],
                            wt[k][:, W_Q[e] : W_Q[e] + SB],
                            hst[k][:, tcc * TC : (tcc + 1) * TC],
                            start=(k == 0),
                            stop=(k == NKA - 1),
                        )
                    nc.scalar.copy(
                        qt[e][:, tcc * TC : (tcc + 1) * TC], pp[:, 0:TC]
                    )

                def emit_k(e, scc):
                    pp = proj_tile()
                    for k in range(NKA):
                        nc.tensor.matmul(
                            pp[:, 0:TC],
                            wt[k][:, W_K[e] : W_K[e] + SB],
                            hst[k][:, scc * TC : (scc + 1) * TC],
                            start=(k == 0),
                            stop=(k == NKA - 1),
                        )
                    nc.scalar.copy(
                        kt[e][:, scc * TC : (scc + 1) * TC], pp[:, 0:TC]
                    )

                def emit_v(sb):
                    pp = proj_tile()
                    for k in range(NKA):
                        nc.tensor.matmul(
                            pp[:, 0:VW],
                            hst[k][:, sb * SB : (sb + 1) * SB],
                            wt[k][:, 256 : 256 + VW],
                            start=(k == 0),
                            stop=(k == NKA - 1),
                        )
                    nc.scalar.copy(vt[sb][:], pp[:, 0:VW])
                    if not with_bias:
                        # ones-selector columns never got their 1.0 (it lives
                        # in the elided bias row): write them directly.
                        for j in range(NHC):
                            nc.vector.memset(
                                vt[sb][:, j * (HD + 1) + HD :
                                       j * (HD + 1) + HD + 1].bitcast(F32),
                                1.0,
                            )

                def emit_attn(pair, tci):
                    sbs = _alive_sbs(tci)
                    pv_acc = [
                        pvpool.tile([HD + 1, TC], F32, tag="pvacc", name="pvacc")
                        for _ in range(2)
                    ]
                    for i, sb in enumerate(sbs):
                        xlo = _x_lo(sb, tci)
                        first, last = (i == 0), (i == len(sbs) - 1)
                        pqk = mmps.tile([SB, 2 * TC], F32, tag="mm", name="pqk")
                        for h2 in range(2):
                            nc.tensor.matmul(
                                pqk[:, h2 * TC + xlo : (h2 + 1) * TC],
                                kt[pair][h2 * HD : (h2 + 1) * HD,
                                         sb * SB : (sb + 1) * SB],
                                qt[pair][h2 * HD : (h2 + 1) * HD,
                                         tci * TC + xlo : (tci + 1) * TC],
                                start=True,
                                stop=True,
                                tile_position=(h2 * HD, 0),
                            )
                        te = epool.tile([SB, 2 * TC], F32R, tag="te", name="te")
                        te3 = te[:].rearrange("p (b n) -> p b n", b=2)
                        pqk3 = pqk[:].rearrange("p (b n) -> p b n", b=2)
                        nc.scalar.activation(
                            te3[:, :, xlo:TC],
                            pqk3[:, :, xlo:TC],
                            EXP,
                            bias=attc[:, sb : sb + 1],
                            scale=SCALE,
                        )
                        for blk, mi, coff in _diag_actions(sb, tci):
                            x = blk * SB + coff
                            m_ap = bass_mod.AP(
                                tensor=aux[:].tensor,
                                offset=NSB + mi * SB + coff,
                                ap=[[AUXW, SB], [0, 2], [1, SB - coff]],
                            )
                            nc.gpsimd.tensor_mul(
                                te3[:, :, x : blk * SB + SB],
                                te3[:, :, x : blk * SB + SB],
                                m_ap,
                            )
                        for h2 in range(2):
                            hl = 2 * pair + h2
                            nc.tensor.matmul(
                                pv_acc[h2][:, xlo:TC],
                                vt[sb][:, hl * (HD + 1) : (hl + 1) * (HD + 1)],
                                te[:, h2 * TC + xlo : (h2 + 1) * TC],
                                start=first,
                                stop=last,
                            )
                    # copy psum->SBUF first (releases pvacc banks ~2.5us
                    # earlier than normalizing straight from PSUM), then
                    # pair-combined normalize: 1 bcast DMA + 1 output DMA
                    pvc = wpool.tile([HD + 1, 2 * TC], F32, tag="pvc",
                                     name="pvc")
                    for h2 in range(2):
                        nc.vector.tensor_copy(
                            pvc[:, h2 * TC : (h2 + 1) * TC], pv_acc[h2][:]
                        )
                    rr2 = wpool.tile([SB, TC], F32, tag="rr", name="rr2")
                    for h2 in range(2):
                        nc.vector.reciprocal(
                            rr2[HD * h2 : HD * h2 + 1, :],
                            pvc[HD : HD + 1, h2 * TC : (h2 + 1) * TC],
                        )
                    rrep2 = wpool.tile([HD, 2 * TC], F32, tag="rrep", name="rrep2")
                    for h2 in range(2):
                        src_b = bass_mod.AP(
                            tensor=rr2[:].tensor,
                            offset=HD * h2 * TC,
                            ap=[[TC, 1], [0, HD], [1, TC]],
                        )
                        nc.scalar.dma_start(
                            out=rrep2[:, h2 * TC : (h2 + 1) * TC], in_=src_b
                        )
                    osb2 = wpool.tile([HD, 2 * TC], F32, tag="osb", name="osb2")
                    for h2 in range(2):
                        nc.vector.tensor_mul(
                            osb2[:, h2 * TC : (h2 + 1) * TC],
                            pvc[0:HD, h2 * TC : (h2 + 1) * TC],
                            rrep2[:, h2 * TC : (h2 + 1) * TC],
                        )
                    for h2 in range(2):
                        hl = 2 * pair + h2
                        x0 = 1 if tci == 1 else 0
                        nc.scalar.dma_start(
                            out=outT_d[hl * HD : (hl + 1) * HD,
                                       tci * TC + x0 : (tci + 1) * TC],
                            in_=osb2[:, h2 * TC + x0 : (h2 + 1) * TC],
                        )

                def emit_special_probs():
                    """t=512 probs row per head -> erT4 [128, 5] PV layout.

                    Head hl lives at partition 32*hl (legal AP base); the
                    normalized probs row reaches PV layout via a DRAM
                    round-trip scatter.  Emitted before pair-1 attention so
                    the whole latency chain hides under it.
                    """
                    for hl in range(NHC):
                        pair, half = hl // 2, hl % 2
                        qcol = qt[pair][half * HD : (half + 1) * HD, 512:513]
                        pr = mmps.tile([SB, 2 * TC], F32, tag="mm", name="pr")
                        nc.tensor.matmul(
                            pr[0:1, 0:512], qcol,
                            kt[pair][half * HD : (half + 1) * HD, 0:512],
                            start=True, stop=True,
                        )
                        nc.tensor.matmul(
                            pr[0:1, 512:513], qcol.bitcast(F32),
                            kt[pair][half * HD : (half + 1) * HD, 512:513]
                            .bitcast(F32),
                            start=True, stop=True,
                        )
                        # attn-mask (host pre-scaled by 1/SCALE)
                        nc.vector.tensor_add(
                            pr[0:1, 0:513], pr[0:1, 0:513], attr[0:1, 0:513]
                        )
                        er = ssb.tile([1, 513], F32, tag=f"er{hl}",
                                      name=f"er{hl}")
                        z = ssb.tile([1, 1], F32, tag=f"z{hl}", name=f"z{hl}")
                        rz = ssb.tile([1, 1], F32, tag=f"rz{hl}",
                                      name=f"rz{hl}")
                        nc.scalar.activation(
                            er[:], pr[0:1, 0:513], EXP,
                            scale=SCALE, accum_out=z[:],
                        )
                        nc.vector.reciprocal(rz[:], z[:])
                        nc.vector.tensor_scalar_mul(er[:], er[:], rz[:])
                        nc.scalar.dma_start(out=scr_d[hl, 0:513], in_=er[:])
                        src_t = bass_mod.AP(
                            tensor=scr_d.ap().tensor,
                            offset=hl * 640,
                            ap=[[1, SB], [SB, 5]],
                        )
                        nc.scalar.dma_start(
                            out=erT4[:, 5 * hl : 5 * hl + 5], in_=src_t
                        )

                def emit_special_pv():
                    po4_t = pvpool.tile([HD + 1, TC], F32, tag="pvacc", name="po4")
                    po4 = po4_t[0:HD, 0:NHC]
                    for hl in range(NHC):
                        for sb4 in range(4):
                            nc.tensor.matmul(
                                po4[:, hl : hl + 1],
                                vt[sb4][:, hl * (HD + 1) : hl * (HD + 1) + HD]
                                .bitcast(F32),
                                erT4[:, 5 * hl + sb4 : 5 * hl + sb4 + 1],
                                start=(sb4 == 0),
                                stop=False,
                            )
                        nc.tensor.matmul(
                            po4[:, hl : hl + 1],
                            vt[4][0:1, hl * (HD + 1) : hl * (HD + 1) + HD]
                            .bitcast(F32),
                            erT4[0:1, 5 * hl + 4 : 5 * hl + 5],
                            start=False,
                            stop=True,
                        )
                    o54 = ssb.tile([HD, NHC], F32, tag="o54", name="o54")
                    nc.vector.tensor_copy(o54[:], po4)
                    for hl in range(NHC):
                        nc.scalar.dma_start(
                            out=outT_d[hl * HD : (hl + 1) * HD, 512:513],
                            in_=o54[:, hl : hl + 1],
                        )

                erT4 = ssb.tile([SB, 5 * NHC], F32, tag="erT4", name="erT4")

                if emission == "front":
                    emit_q(0, 0); emit_k(0, 0)
                    for sb in range(4):
                        emit_v(sb)
                    emit_attn(0, 0)
                    emit_q(0, 1); emit_k(0, 1)
                    for sb in range(4, 8):
                        emit_v(sb)
                    emit_attn(0, 1)
                    emit_q(0, 2); emit_k(0, 2)
                    for sb in range(8, NVT):
                        emit_v(sb)
                    emit_attn(0, 2)
                    emit_q(0, 3); emit_q(1, 0); emit_k(1, 0)
                    emit_attn(0, 3)
                    emit_q(1, 1); emit_q(1, 2); emit_q(1, 3)
                    emit_k(1, 1); emit_k(1, 2)
                    if not no_special:
                        emit_special_probs()
                    emit_attn(1, 0); emit_attn(1, 1)
                    emit_attn(1, 2); emit_attn(1, 3)
                    if not no_special:
                        emit_special_pv()
                else:  # paced
                    emit_q(0, 0); emit_k(0, 0)
                    for sb in range(4):
                        emit_v(sb)
                    emit_attn(0, 0)
                    emit_q(0, 1)
                    emit_attn(0, 1)
                    emit_q(0, 2); emit_k(0, 1)
                    for sb in range(4, 8):
                        emit_v(sb)
                    emit_attn(0, 2)
                    emit_q(0, 3); emit_k(0, 2)
                    for sb in range(8, NVT):
                        emit_v(sb)
                    emit_q(1, 0); emit_k(1, 0)
                    emit_attn(0, 3)
                    emit_q(1, 1); emit_q(1, 2)
                    emit_attn(1, 0)
                    emit_q(1, 3); emit_k(1, 1)
                    emit_attn(1, 1)
                    emit_k(1, 2)
                    emit_attn(1, 2)
                    emit_attn(1, 3)
                    if not no_special:
                        emit_special_probs()
                        emit_special_pv()

    nc.compile()
    return nc


def _host_prep(inputs, with_bias):
    hs = np.asarray(inputs["hidden_states"], dtype=np.float32)
    am = np.asarray(inputs["attention_mask"], dtype=np.float32)
    Wq = np.asarray(inputs["Wq"], dtype=np.float32)
    bq = np.asarray(inputs["bq"], dtype=np.float32)
    Wk = np.asarray(inputs["Wk"], dtype=np.float32)
    bk = np.asarray(inputs["bk"], dtype=np.float32)
    Wv = np.asarray(inputs["Wv"], dtype=np.float32)
    bv = np.asarray(inputs["bv"], dtype=np.float32)

    p = np.arange(SB)[:, None]
    x = np.arange(SB)[None, :]
    m0 = (p <= x).astype(np.float32)
    m1 = (p <= x - 1).astype(np.float32)
    masks = np.concatenate([m0, m1], axis=1)
    eye4 = np.zeros((SB, 4), dtype=np.float32)
    eye4[:4] = np.eye(4, dtype=np.float32)

    in_maps = []
    for c in range(8):
        b, g = c // 4, c % 4
        hsT_aug = np.concatenate(
            [hs[b].T, np.ones((1, S), dtype=np.float32)], axis=0
        )
        # w cols: [q-e0 | k-e0 | v_aug | q-e1 | k-e1]
        w = np.zeros((HP, WCOLS), dtype=np.float32)
        hsl = slice(256 * g, 256 * (g + 1))
        WqT = Wq[hsl, :].T
        WkT = Wk[hsl, :].T
        w[:H, 0:128] = WqT[:, 0:128]
        w[H, 0:128] = bq[hsl][0:128]
        w[:H, 516:644] = WqT[:, 128:256]
        w[H, 516:644] = bq[hsl][128:256]
        w[:H, 128:256] = WkT[:, 0:128]
        w[H, 128:256] = bk[hsl][0:128]
        w[:H, 644:772] = WkT[:, 128:256]
        w[H, 644:772] = bk[hsl][128:256]
        for j in range(NHC):
            cs = slice(256 * g + HD * j, 256 * g + HD * (j + 1))
            w[:H, 256 + 65 * j : 256 + 65 * j + HD] = Wv[cs, :].T
            w[H, 256 + 65 * j : 256 + 65 * j + HD] = bv[cs]
            w[H, 256 + 65 * j + HD] = 1.0  # ones-selector column
        amv = am[b, 0, 0, :].astype(np.float32)
        attc = amv.reshape(NSB, SB).T
        attr_row = np.zeros((SB, S), dtype=np.float32)
        attr_row[0] = amv / SCALE
        aux = np.concatenate([attc, masks, eye4, attr_row], axis=1)
        in_maps.append({"hst": hsT_aug, "w": w, "aux": aux.copy()})
    return in_maps


LAST_EXEC_NS = None


def kernel(**inputs):
    import os

    from concourse.bass_utils import run_bass_kernel_spmd

    global LAST_EXEC_NS
    with_bias = bool(
        np.any(np.asarray(inputs["bq"]))
        or np.any(np.asarray(inputs["bk"]))
        or np.any(np.asarray(inputs["bv"]))
    )
    key = f"nc{int(with_bias)}"
    if key not in _CACHE:
        _CACHE[key] = _build_program(with_bias=with_bias)
    nc = _CACHE[key]
    in_maps = _host_prep(inputs, with_bias)
    trace = bool(os.environ.get("BASS_KERNEL_TRACE"))
    res = run_bass_kernel_spmd(nc, in_maps, list(range(8)), trace=trace)
    LAST_EXEC_NS = res.exec_time_ns
    out = np.empty((B, S, H), dtype=np.float32)
    for c in range(8):
        b, g = c // 4, c % 4
        out[b, :, 256 * g : 256 * (g + 1)] = res.results[c]["outT"].T
    return out
